# revision 44
# baseline (speedup 1.0000x reference)
"""Trainium2 Bass kernel for nn_Attention (dense transformer block).

Reference computation per batch image (B=8, H=W=64, C=192, D=24, L=4096):
    q = x @ w_q; k = x @ w_k; v = x @ w_v          # [L, D]
    s = q @ k^T                                    # [L, L]
    beta = softmax(s, axis=-1)
    out = gamma * (beta @ v) @ w_o + x             # [L, C]

Sharding: pure data parallel, one image per NeuronCore (8 cores).

Per-core dataflow (matmuls bf16, fp32 PSUM accumulate). The PE array is
packed 4x both ways since the head dim (24) wastes the 128x128 array:
  - x^T arrives pre-transposed (bf16) from the host (pure layout transform).
  - scores are row-tiled: 4 key chunks computed concurrently in row groups
    32g against group-stacked q^T/k^T [128, L] tiles built by one matmul
    with host-side stacked weights [C, 128]. Each row group's output goes
    to its own PSUM bank (HW requirement for row tiling).
  - softmax exp is split across ScalarE (exact, ACTIVATE) and VectorE
    (Schraudolph bf16 bit-trick: one fused mult+add with int16 convert),
    keeping both engines ~equally busy.
  - attention accumulation is col-tiled: vhat chunks (M=32: v | ones | 0pad)
    accumulate into group 32g of the stacked partials [128, W]; a K=1
    zeroing matmul arms has_written for all partitions first.
  - v itself is never projected separately: w_v rides in spare zero columns
    of wk_stack, and v^T strips take a DRAM round trip through the xbar
    DMA transpose to become token-major vhat (zero PE cost). The ones/zeros
    rows of the vt scratch are pre-seeded by the host (vt is an input).
  - epilogue (chunk-PAIR batched): two token chunks share one 1-bank PSUM
    tile (2 matmuls), ONE plain ScalarE copy per pair amortizes the
    352-cycle ACT pipe-fill, a strided DVE reciprocal covers both
    denominators, and the per-token divide + residual add run on the
    otherwise-idle GpSimd (broadcast-AP tensor_mul + tensor_add; GpSimd
    rejects AP-scalar TensorScalarPtr at codegen). The last window's
    divides run on DVE instead so the tail chain pipelines across engines.
  - attention emission is deferred THREE quads behind the score stream so
    both exp tiles (ScalarE + the lagging DVE) are complete when the four
    col-tiled attn MMs issue; the es pool is 20 deep to cover the skew.
  - the steady state is PE-stream-bound at 1.2 GHz (HAM re-throttles on
    the pipeline's micro-gaps and never recovers; measured MM walls are
    clock-invariant): cadence ~2.0us per key-quad = one [128,1024]-col
    exp per engine + 4-conc score group + 4-conc attn group + amortized
    projection/epilogue MMs. DMA: xt split across both HWDGE rings,
    weights on the GpSimd SWDGE queue, x residual + v^T round trip +
    outputs on the sync ring ordered so nothing queues behind the 3.1MB
    x load.

Softmax w/o max subtraction is safe: scores range ~[-50, 54];
exp(54) ~ 2e23 << fp32/bf16 max; row sums < 1e27.
"""

import numpy as np

import concourse.bass as bass
import concourse.tile as tile
from concourse import bacc, mybir
from concourse.bass_utils import run_bass_kernel_spmd

F32 = mybir.dt.float32
BF16 = mybir.dt.bfloat16

B = 8
L = 4096          # tokens per image (64*64)
C = 192           # channels
D = 24            # head dim (q/k/v)
G = 4             # PE array packing groups
NCH = L // 128    # 32 chunks of 128 tokens
W = 512           # i-window (moving free dim per matmul)
NIW = L // W      # 8 i-windows
WIN = 512         # projection window (rhs free dim)
NWIN = L // WIN   # 8 windows
NQ = NCH // G     # 8 quads of key chunks
VW = 32           # padded vhat chunk width (v | ones | zeros)


def build_graph():
    """Build the single-core Bass graph (SPMD: identical on all 8 cores)."""
    nc = bacc.Bacc(
        "TRN2", target_bir_lowering=False, debug=False, num_devices=8,
        name="attn_dp",
    )

    x_ext = nc.dram_tensor("x", [L, C], F32, kind="ExternalInput").ap()
    xt_ext = nc.dram_tensor("xt", [C, L], BF16, kind="ExternalInput").ap()
    # group-stacked projection weights [C, 128]: col 32g+d = w[:, d]
    wqs_ext = nc.dram_tensor("wq_stack", [C, 128], F32,
                             kind="ExternalInput").ap()
    wks_ext = nc.dram_tensor("wk_stack", [C, 128], F32,
                             kind="ExternalInput").ap()
    # wo_stack [128, 193]: rows 32g+d = gamma * w_o[d]; rows 32g+24 col 192 = 1
    wos_ext = nc.dram_tensor("wo_stack", [128, C + 1], F32,
                             kind="ExternalInput").ap()
    # v^T scratch; host pre-seeds rows 24:32 with the ones/zeros block so the
    # kernel only has to fill rows 0:24 (the actual v^T strips) at runtime.
    vt_ext = nc.dram_tensor("vt", [VW, L], BF16, kind="ExternalInput").ap()
    out_ext = nc.dram_tensor("out", [L, C], F32, kind="ExternalOutput").ap()

    with tile.TileContext(nc) as tc:
        _build(tc, x_ext, xt_ext, wqs_ext, wks_ext, wos_ext, vt_ext, out_ext)

    nc.compile()
    return nc


def _build(tc, x_ext, xt_ext, wqs_ext, wks_ext, wos_ext, vt_ext, out_ext):
    nc = tc.nc

    with (
        # ---- persistent SBUF ----
        tc.tile_pool(name="const", bufs=1) as const_pool,
        tc.tile_pool(name="xsb", bufs=1) as x_pool,
        tc.tile_pool(name="xT", bufs=1) as xT_pool,
        tc.tile_pool(name="qkT", bufs=1) as qkT_pool,
        tc.tile_pool(name="vhat", bufs=1) as vhat_pool,
        tc.tile_pool(name="expS", bufs=20) as expS_pool,
        tc.tile_pool(name="pt", bufs=2) as pt_pool,
        tc.tile_pool(name="outst", bufs=4) as outst_pool,
        tc.tile_pool(name="epst", bufs=3) as eps_pool,
        tc.tile_pool(name="rden", bufs=4) as r_pool,
        # ---- PSUM (8 banks): scores/proj/epilogue 3x2 + partials 2x1 ----
        tc.tile_pool(name="ps_s", bufs=3, space="PSUM") as ps_scores,
        tc.tile_pool(name="ps_acc", bufs=2, space="PSUM") as ps_partials,
    ):
        # ================= PE warm-up (issue ASAP: HAM un-throttle) ========
        # single DVE memset; no GpSimd serialization in front of the PE.
        # HAM dies once the steady state starts, but 8/8 through the prologue
        # makes the 16 projection MMs ~2x faster.
        warm = const_pool.tile([128, 384], BF16)
        nc.vector.memset(warm[:], 0.0)
        warm_ps = ps_scores.tile([128, 256], F32, tag="s", name="warm_ps")
        for _ in range(25):
            nc.tensor.matmul(warm_ps[:], warm[:, 0:128], warm[:, 128:384],
                             start=True, stop=True)

        # zeros for the partials-bank init matmul (K=1): out = zl.T @ zr = 0
        zl = const_pool.tile([1, 128], BF16)
        zr = const_pool.tile([1, W], BF16)
        nc.vector.memset(zl[:], 0.0)
        nc.vector.memset(zr[:], 0.0)

        # ================= x / x^T / weight loads =================
        x_sb = x_pool.tile([128, NCH * C], F32)       # chunk c at cols [C*c, ...)
        xTa = xT_pool.tile([128, L], BF16)            # x^T rows 0..127 (channels)
        xTb = xT_pool.tile([64, L], BF16)             # x^T rows 128..191

        # xt windows split across the two HWDGE rings so neither serializes
        # the projection stream; weights ride the SWDGE (GpSimd) queue
        def xt_window(w):
            sl = slice(WIN * w, WIN * (w + 1))
            # balance bytes per window across the two rings: split the a-half
            # (128 rows) 64/64 and alternate which ring takes the b-half
            ra, rb = (nc.sync, nc.scalar) if w % 2 == 0 else (nc.scalar, nc.sync)
            ra.dma_start(xTa[0:64, sl], xt_ext[0:64, sl])
            rb.dma_start(xTa[64:128, sl], xt_ext[64:128, sl])
            ra.dma_start(xTb[:, sl], xt_ext[128:192, sl])

        wstage = const_pool.tile([128, 760], F32)  # fp32 staging for weights
        def load_weight_bf(ext, rows, cols, stage_off, tag):
            st = wstage[:rows, stage_off:stage_off + cols]
            nc.gpsimd.dma_start(st, ext)
            t = const_pool.tile([rows, cols], BF16, tag=tag)
            nc.vector.tensor_copy(t[:], st)
            return t

        SR = 128
        wksa = load_weight_bf(wks_ext[0:128, :], 128, SR, 2 * SR, "wksa")
        wksb = load_weight_bf(wks_ext[128:192, :], 64, SR, 3 * SR, "wksb")
        wqsa = load_weight_bf(wqs_ext[0:128, :], 128, SR, 0, "wqsa")
        wqsb = load_weight_bf(wqs_ext[128:192, :], 64, SR, SR, "wqsb")
        wos = load_weight_bf(wos_ext, 128, C + 1, 4 * SR, "wos")

        for w in range(NWIN):
            xt_window(w)

        kTs = qkT_pool.tile([128, L], BF16)           # stacked k^T replicas
        qTs = qkT_pool.tile([128, L], BF16)           # stacked q^T replicas
        vhat = vhat_pool.tile([128, NCH * VW], BF16)  # v | ones | zero pad
        vhat_view = vhat.rearrange("p (j d) -> p j d", d=VW)

        def project(dst, wa, wb, w, nm, eng=None, pool=None):
            ps = (pool or ps_scores).tile([128, WIN], F32, tag="s" if pool is None else "acc",
                                          name=f"pj{nm}{w}")
            sl = slice(WIN * w, WIN * (w + 1))
            nc.tensor.matmul(ps[:], wa[:], xTa[:, sl], start=True, stop=False)
            nc.tensor.matmul(ps[:], wb[:], xTb[:, sl], start=False, stop=True)
            if eng == "act":
                # split the copy across ScalarE and DVE so the prologue's
                # per-window copy chain runs in parallel on both engines
                h = WIN // 2
                nc.scalar.copy(dst[:, sl.start:sl.start + h], ps[:, 0:h])
                nc.vector.tensor_copy(dst[:, sl.start + h:sl.stop], ps[:, h:])
            else:
                nc.vector.tensor_copy(dst[:, sl], ps[:])

        pt_tiles = {}

        def emit_epilogue(piw, sp):
            # one pair of 128-token chunks per call: 2 matmuls into one
            # 1-bank PSUM tile, ONE plain ScalarE copy (no per-chunk ACT
            # pipe-fill tax), strided reciprocal for both denominators, and
            # the per-token divide + residual add fused on GpSimd
            ptb = pt_tiles[piw]
            ep2 = ps_scores.tile([128, 2 * (C + 1)], F32, tag="s",
                                 name=f"ep{piw}_{sp}")
            for k in range(2):
                s = 2 * sp + k
                nc.tensor.matmul(ep2[:, (C + 1) * k:(C + 1) * (k + 1)],
                                 ptb[:, 128 * s:128 * (s + 1)],
                                 wos[:], start=True, stop=True)
            rr2 = r_pool.tile([128, 2], F32, name=f"rr{piw}_{sp}", tag="rr")
            ep2v = ep2.rearrange("p (two c) -> p two c", c=C + 1)
            nc.vector.reciprocal(rr2[:], ep2v[:, :, C])
            eps = eps_pool.tile([128, 2 * (C + 1)], F32,
                                name=f"eps{piw}_{sp}", tag="eps")
            nc.scalar.copy(eps[:], ep2[:])
            for k in range(2):
                cidx = (W // 128) * piw + 2 * sp + k
                ot = outst_pool.tile([128, C], F32, name=f"ot{piw}_{sp}_{k}",
                                     tag="ot")
                if piw == NIW - 1:
                    # tail: the serial GpSimd mul+add chain would be exposed;
                    # pipeline the divide on the (now idle) DVE instead
                    nc.vector.tensor_scalar_mul(
                        ot[:], eps[:, (C + 1) * k:(C + 1) * k + C],
                        rr2[:, k:k + 1])
                else:
                    nc.gpsimd.tensor_mul(
                        ot[:], eps[:, (C + 1) * k:(C + 1) * k + C],
                        rr2[:, k:k + 1].broadcast_to([128, C]))
                nc.gpsimd.tensor_add(ot[:], ot[:],
                                     x_sb[:, C * cidx:C * (cidx + 1)])
                nc.sync.dma_start(out_ext[128 * cidx:128 * (cidx + 1), :],
                                  ot[:])

        # bulk prologue: k projections (ScalarE copies — DVE is the steady-
        # state bottleneck); v^T strips exported per-half so the first attn
        # quads never wait on the full round trip
        def vt_export(h):
            sl = slice(2048 * h, 2048 * (h + 1))
            nc.sync.dma_start(vt_ext[0:8, sl], kTs[24:32, sl])
            nc.sync.dma_start(vt_ext[8:16, sl], kTs[56:64, sl])
            nc.sync.dma_start(vt_ext[16:24, sl], kTs[88:96, sl])
            nc.sync.dma_start_transpose(out=vhat_view[:, 16 * h:16 * (h + 1), :],
                                        in_=vt_ext[:, sl])

        # q-proj for window 0 rides right after the first k-window: score
        # quad t only depends on k-window t, so the main loop's early quads
        # overlap the remaining k-projections
        for w in range(NWIN):
            project(kTs, wksa, wksb, w, "k", eng="act")
            if w == 0:
                project(qTs, wqsa, wqsb, 0, "q")
            if w == 3:
                vt_export(0)
        vt_export(1)

        # x (residual input) rides the GpSimd SWDGE queue: on the sync ring
        # the scheduler hoists these dep-free 3.1MB descriptors ahead of the
        # v^T strips (which wait on the k-copies) and queue-blocks the
        # second vhat transpose until ~38us, stalling the attention stream
        x_src = x_ext.rearrange("(c p) j -> p c j", p=128)
        x_dst = x_sb[:].rearrange("p (c j) -> p c j", j=C)
        for i in range(8):
            nc.gpsimd.dma_start(x_dst[:, 4 * i:4 * (i + 1), :],
                                x_src[:, 4 * i:4 * (i + 1), :])

        # ================= main loop =================
        # i-windows of W=512; key chunks in quads of 4 (row groups 0..3).
        # Each row group's scores land in a distinct PSUM bank (HW rule):
        # groups 0/1 -> scA banks 0/1, groups 2/3 -> scB banks 0/1.
        # The epilogue of window iw-1 is deferred into iw's quad loop so the
        # inter-window dependency chain never stalls the exp stream.
        def emit_attn(partials_, t, ess):
            for g in (0, 1, 2, 3):
                j = G * t + g
                nc.tensor.matmul(
                    partials_[32 * g:32 * g + VW, :],
                    vhat[:, VW * j:VW * (j + 1)],
                    ess[g // 2][:, 512 * (g % 2):512 * (g % 2 + 1)],
                    start=False, stop=(t == NQ - 1),
                    tile_position=(0, 32 * g),
                    skip_group_check=True,
                )

        # attention is deferred TWO quads so both exp tiles (ScalarE h0 and
        # the chronically-lagging DVE h1) are complete when the attn MMs
        # issue -> all four col-tiled MMs launch concurrently instead of as
        # two staggered pairs. The deque spans window boundaries; ptb copy /
        # epilogue chunks chase the deferred stream.
        from collections import deque
        attn_q = deque()
        NSP = W // 256  # chunk-pairs per window
        state = {"pending": None, "chunk": NSP}

        def flush_epilogue():
            while state["chunk"] < NSP:
                if state["chunk"] >= 0:
                    emit_epilogue(state["pending"], state["chunk"])
                state["chunk"] += 1

        def pop_attn():
            partials_, iw_, t_, ess_ = attn_q.popleft()
            emit_attn(partials_, t_, ess_)
            if t_ == NQ - 1:
                ptb = pt_pool.tile([128, W], BF16, name=f"ptb{iw_}", tag="ptb")
                nc.scalar.copy(ptb[:], partials_[:])
                pt_tiles[iw_] = ptb
                flush_epilogue()
                state["pending"] = iw_
                # -1: skip one slot before the first epilogue pair so its MM
                # never head-of-line-blocks the PE FIFO while the ptb copy
                # (fresh off the deferred attention chain) completes
                state["chunk"] = -1

        for iw in range(NIW):
            isl = slice(W * iw, W * (iw + 1))
            partials = ps_partials.tile([128, W], F32, name=f"partials{iw}",
                                        tag="acc")
            # zero-init the bank and set has_written on all 128 partitions so
            # the col-tiled accumulating matmuls below can all use start=False
            nc.tensor.matmul(partials[:, :], zl[:], zr[:],
                             start=True, stop=False, skip_group_check=True)
            for t in range(NQ):
                # pop the deferred attention FIRST: it is dependency-free by
                # now and fills the PE while this quad's scores wait on the
                # exp/PSUM rotation; trailing the small epilogue MM instead
                # of a score-quad drain also frees XBUS budget for 4-wide
                # col-tiled launch
                if len(attn_q) == 3:
                    pop_attn()
                scs = [ps_scores.tile([128, 1024], F32, tag="s", name=f"sc{iw}_{t}_0"),
                       ps_scores.tile([128, 1024], F32, tag="s", name=f"sc{iw}_{t}_1")]
                for g in range(G):
                    j = G * t + g
                    nc.tensor.matmul(
                        scs[g // 2][:, 512 * (g % 2):512 * (g % 2 + 1)],
                        kTs[32 * g:32 * g + 32, 128 * j:128 * (j + 1)],
                        qTs[32 * g:32 * g + 32, isl],
                        start=True, stop=True,
                        tile_position=(32 * g, 0),
                    )
                ess = []
                for h in range(2):
                    es = expS_pool.tile([128, 1024], BF16, name=f"es{iw}_{t}_{h}", tag="es")
                    if h == 1:
                        # Schraudolph exp on DVE: bf16 bits = round(s*log2e*128
                        # + 127*128); one fused mult+add with int16 convert
                        nc.vector.tensor_scalar(
                            es[:].bitcast(mybir.dt.int16), scs[h][:],
                            184.66496580927026, 16256.0,
                            op0=mybir.AluOpType.mult, op1=mybir.AluOpType.add)
                    else:
                        nc.scalar.activation(es[:], scs[h][:],
                                             mybir.ActivationFunctionType.Exp)
                    ess.append(es)
                if t == 0 and iw + 1 < NIW:
                    # prefetch next window's q^T early; its PSUM tile comes
                    # from the partials pool (the slot opposite the live
                    # window is free) so it never stalls the score stream
                    project(qTs, wqsa, wqsb, iw + 1, "q", pool=ps_partials)
                if state["pending"] is not None and state["chunk"] < NSP:
                    if state["chunk"] >= 0:
                        emit_epilogue(state["pending"], state["chunk"])
                    state["chunk"] += 1
                attn_q.append((partials, iw, t, ess))

        while attn_q:
            pop_attn()
        flush_epilogue()


_CACHE = {}


def _get_graph():
    if "nc" not in _CACHE:
        _CACHE["nc"] = build_graph()
    return _CACHE["nc"]


def make_in_maps(tensor, w_q, w_k, w_v, w_o, gamma):
    import ml_dtypes
    x = np.ascontiguousarray(np.asarray(tensor, dtype=np.float32)).reshape(B, L, C)
    xt = np.ascontiguousarray(
        x.transpose(0, 2, 1).astype(ml_dtypes.bfloat16))  # [B, C, L] bf16
    wq = np.asarray(w_q, dtype=np.float32)
    wk = np.asarray(w_k, dtype=np.float32)
    wv = np.ascontiguousarray(np.asarray(w_v, dtype=np.float32))
    wo = np.asarray(w_o, dtype=np.float32)

    wq_stack = np.zeros((C, 128), dtype=np.float32)
    wk_stack = np.zeros((C, 128), dtype=np.float32)
    for g in range(G):
        wq_stack[:, 32 * g:32 * g + D] = wq
        wk_stack[:, 32 * g:32 * g + D] = wk
    # w_v rides in the spare zero columns of wk_stack (contracts against
    # zero rows of the q stack, so scores are unaffected); the k-projection
    # then produces v^T rows for free.
    wk_stack[:, 24:32] = wv[:, 0:8]
    wk_stack[:, 56:64] = wv[:, 8:16]
    wk_stack[:, 88:96] = wv[:, 16:24]

    wo_stack = np.zeros((128, C + 1), dtype=np.float32)
    for g in range(G):
        wo_stack[32 * g:32 * g + D, :C] = wo * np.float32(gamma)
        wo_stack[32 * g + D, C] = 1.0

    # vt scratch pre-seed: rows 0:24 are overwritten at runtime with the v^T
    # strips; rows 24:32 carry the constant ones/zeros block (row 24 = 1.0)
    vt0 = np.zeros((VW, L), dtype=ml_dtypes.bfloat16)
    vt0[24, :] = 1.0

    return [
        {"x": np.ascontiguousarray(x[b]), "xt": xt[b], "wq_stack": wq_stack,
         "wk_stack": wk_stack, "wo_stack": wo_stack, "vt": vt0}
        for b in range(B)
    ]


def kernel(tensor, w_q, w_k, w_v, w_o, gamma):
    nc = _get_graph()
    in_maps = make_in_maps(tensor, w_q, w_k, w_v, w_o, gamma)
    res = run_bass_kernel_spmd(nc, in_maps, core_ids=list(range(B)))
    out = np.stack([np.asarray(res.results[b]["out"]) for b in range(B)])
    return out.reshape(B, 64, 64, C).astype(np.float32)
